# revision 47
# baseline (speedup 1.0000x reference)
"""Trainium2 Bass kernel for DepthSepConv2d (depthwise 3x3 reflect-pad conv +
sync-BN + ReLU + 1x1 conv + sync-BN + ReLU), data-parallel over batch on 8
NeuronCores.

Self-contained: hardcodes all shapes; host-side code reflect-pads + converts
to bf16, runs the SPMD NEFF, and reassembles the f32 output.

Per core (4 images, 256ch in / 512ch out, 56x56):
  P1: depthwise 3x3 — 5 units on PE (per-tap diagonal matmuls into PSUM,
      2-bank-group ACT evictions with accum stats), 3 units on DVE using
      FLAT shifted taps: all 9 taps are flat step-1 bf16 STT ops over the
      whole padded image (hits the DVE 2x packed mode; pad positions carry
      garbage that is never read), with a column-shifted copy of x supplying
      4-byte-aligned sources for odd shifts. ACT extracts the valid window
      (+BN sum); squares run on GPSIMD.
  AR1a/AR1b: split sync-BN all-reduce; AR1a (7 units) hides under the tail
      unit's compute; one warmup collective at t=0 absorbs core launch skew.
  P2: yh = relu(a1*y+c1) computed IN PLACE over y on ACT; 1x1 GEMM from the
      same tiles; one batched 4-bank PSUM eviction per tile on DVE (no
      accumulators: BN2 channel sums come from a tiny f32 matmul on the yh
      row sums); squares split GPSIMD/DVE/ACT. s-matmuls emitted mid-GEMM
      so AR2a hides under the last image's GEMM.
  P3: ACT/DVE normalize + ReLU, bf16 output DMA (host converts to f32).
"""

import contextlib

import numpy as np

from concourse import bacc, mybir, tile
from concourse.bass_utils import run_bass_kernel_spmd

N_CORES = 8
B, C1, C2, H, W = 32, 256, 512, 56, 56
BL = B // N_CORES            # images per core
PX = H * W                   # 3136
HP, WD = H + 2, W + 4        # padded rows 58, padded row width 60
XF = HP * WD                 # 3480 flat padded image
XFP = XF + 4                 # flat tile size (tap reads run 2 past the end)
YF = 56 * WD                 # 3360 flat tap-output rows
NN = 3358                    # flat tap span (covers extracted region)
QW = 448                     # pixel tile (8 rows), 7 per image
NQ = PX // QW                # 7
NCB1 = C1 // 128             # 2
NCB2 = C2 // 128             # 4
COUNT = B * PX               # global BN reduction count
EPS = 1e-5

F32 = mybir.dt.float32
BF16 = mybir.dt.bfloat16
AF = mybir.ActivationFunctionType
ALU = mybir.AluOpType
AX = mybir.AxisListType

TAPS = [(dh, dw) for dh in range(3) for dw in range(3)]

# unit = (img, cb); stats slot u = cb*BL + img (cb-major so the partial
# all-reduce slices contiguously; the tail unit must own the last slot).
DVE_UNITS = [(0, 0), (0, 1)]
PE_UNITS = [(1, 0), (1, 1), (2, 0), (2, 1), (3, 0), (3, 1)]
N_DVE = len(DVE_UNITS)

QGROUPS = [(0, 1), (2, 3), (4, 5), (6,)]

# engine choices: 'v' = DVE, 'a' = ACT, 'g' = GPSIMD (copies only).
# Emission order follows expected y completion so queues never head-block;
# the tail unit's square stays on ACT so it cannot delay AR1a.
SQ1_ENG = {u: 'a' for u in DVE_UNITS + PE_UNITS}
SQ1_ENG[(0, 0)] = 'v'
SQ1_ENG[(0, 1)] = 'v'
SQ1_ORDER = [(1, 0), (1, 1), (0, 0), (2, 0), (2, 1), (0, 1), (3, 0), (3, 1)]
SQ2_PAT = ['v', 'a', 'v', 'a', 'a', 'v', 'a', 'v',
           'v', 'a', 'v', 'a']                 # per (img, ob), img 0..2
SQ3_BIG = ['v', 'a', 'v', 'a']                 # img3 main chunk (q0..5)
SQ3_TAIL = ['a', 'v', 'a', 'v']                # img3 tail chunk (q6)
# yh engine per (img, cb): DVE covers the early images (it idles at P2
# start), keeping the ACT chain short enough that GEMM never waits.
YH_ENG = {(0, 0): 'a', (0, 1): 'v', (1, 0): 'v', (1, 1): 'a',
          (2, 0): 'a', (2, 1): 'a', (3, 0): 'a', (3, 1): 'a'}
P3_PAT = ['a', 'v'] * 8


def build():
    nc = bacc.Bacc(None, target_bir_lowering=False, debug=False)

    x_ext = nc.declare_dram_parameter("x", [BL, NCB1, 128, HP, WD], BF16, isOutput=False)
    diag_ext = nc.declare_dram_parameter("diag", [128, NCB1, 9, 128], BF16, isOutput=False)
    dwt_ext = nc.declare_dram_parameter("dwt", [128, NCB1, 9], F32, isOutput=False)
    wtb_ext = nc.declare_dram_parameter("wtb", [128, NCB1, C2], BF16, isOutput=False)
    wtf_ext = nc.declare_dram_parameter("wtf", [128, NCB1, C2], F32, isOutput=False)
    par_ext = nc.declare_dram_parameter("par", [128, 12], F32, isOutput=False)
    out_ext = nc.declare_dram_parameter("out", [BL, NCB2, 128, PX], BF16, isOutput=True)

    with tile.TileContext(nc) as tc:
        with (
            tc.tile_pool(name="persist", bufs=1) as pp,
            tc.tile_pool(name="dram", bufs=1, space="DRAM") as dram,
        ):
            # ---- persistent tiles ----
            y_t = pp.tile([128, NCB1, BL, H, W], BF16, tag="y")

            dwt_sb = pp.tile([128, NCB1, 9], F32, tag="dwt")
            wtb_sb = pp.tile([128, NCB1, C2], BF16, tag="wtb")
            wtf_sb = pp.tile([128, NCB1, C2], F32, tag="wtf")
            par_sb = pp.tile([128, 12], F32, tag="par")

            s1 = pp.tile([128, 2 * BL, 4], F32, tag="s1")     # dw sums, slot u=cb*BL+img
            q1 = pp.tile([128, 2 * BL + 1], F32, tag="q1")    # dw sumsq; the tail
            # unit splits its square into slots 7 (q0-5) + 8 (q6 tail)
            s2s = pp.tile([128, NCB1, BL + 1], F32, tag="s2s")  # yh row sums
            # (slot 4 = second half of img0, whose yh is split for latency)
            q2 = pp.tile([128, NCB2, 5], F32, tag="q2")       # z sumsq slots:
            # 0..2 = img0..2; 3 = img3 q0-5; 4 = img3 q6 (split so the AR2b
            # inputs are ready ~2us after the last eviction)
            sum2 = pp.tile([128, NCB2], F32, tag="sum2")

            a1 = pp.tile([128, NCB1], F32, tag="a1")
            c1 = pp.tile([128, NCB1], F32, tag="c1")
            a2 = pp.tile([128, NCB2], F32, tag="a2")
            c2 = pp.tile([128, NCB2], F32, tag="c2")
            epsb = pp.tile([128, 1], F32, tag="epsb")

            # ---- collective warmup rings its doorbell first (absorbs
            # cross-core launch skew / CC boot before AR1 needs them) ----
            wsb = pp.tile([128, 4], F32, tag="wsb")
            nc.vector.memset(wsb[:], 0.0)
            w_in = dram.tile([128, 4], F32)
            w_out1 = dram.tile([128, 4], F32, addr_space="Shared")
            nc.sync.dma_start(w_in[:], wsb[:])
            nc.gpsimd.collective_compute(
                "AllReduce", ALU.add,
                replica_groups=[list(range(N_CORES))],
                ins=[w_in[:].opt()], outs=[w_out1[:].opt()],
            )

            # ---- small param loads (x loads follow in the P1 pool; par/wt
            # queue later — they are not needed until finalize/P2) ----
            nc.sync.dma_start(dwt_sb[:], dwt_ext[:])
            nc.vector.memset(epsb[:], EPS)
            nc.vector.memset(s1[:], 0.0)

            # ================= P1: depthwise conv + BN1 stats =================
            with (
                tc.tile_pool(name="p1sb", bufs=1) as p1,
                tc.tile_pool(name="p1ps", bufs=1, space="PSUM") as p1ps,
                nc.named_scope("P1_dwconv"),
            ):
                diag_sb = p1.tile([128, NCB1, 9, 128], BF16, tag="diag")
                nc.sync.dma_start(diag_sb[:], diag_ext[:])

                xp_t = {}

                def emit_load(img, cb, dve):
                    # two half-loads so the first matmuls start ~2us sooner
                    xp = p1.tile([128, HP, WD], BF16,
                                 tag="xpv" if dve else "xp", bufs=2,
                                 name=f"xp_{img}_{cb}")
                    nc.sync.dma_start(xp[:, 0:29, :], x_ext[img, cb, :, 0:29, :])
                    nc.sync.dma_start(xp[:, 29:HP, :], x_ext[img, cb, :, 29:HP, :])
                    xp_t[(img, cb)] = xp

                def emit_pe_unit(img, cb):
                    u = cb * BL + img
                    xp = xp_t[(img, cb)]
                    for g, qs in enumerate(QGROUPS):
                        ps = p1ps.tile([128, 2, 512], F32, tag="dps", bufs=4,
                                       name=f"dps_{img}_{cb}_{g}")
                        for qi, q in enumerate(qs):
                            for t, (dh, dw) in enumerate(TAPS):
                                rhs = xp[:, q * 8 + dh: q * 8 + dh + 8,
                                         dw + 1: dw + 57]
                                nc.tensor.matmul(
                                    ps[:, qi, 0:QW], diag_sb[:, cb, t, :], rhs,
                                    start=(t == 0), stop=(t == 8))
                        r0 = qs[0] * 8
                        nr = len(qs) * 8
                        nc.scalar.activation(
                            y_t[:, cb, img, r0:r0 + nr, :],
                            ps[:, 0:len(qs), 0:QW], AF.Copy,
                            accum_out=s1[:, u, g:g + 1])

                def emit_dve_unit(img, cb):
                    u = cb * BL + img
                    xp = xp_t[(img, cb)]
                    yv = y_t[:, cb, img, :, :]
                    for t, (dh, dw) in enumerate(TAPS):
                        src = xp[:, dh:dh + H, dw + 1:dw + 57]
                        wsc = dwt_sb[:, cb, t:t + 1]
                        if t == 0:
                            nc.vector.tensor_scalar(yv, src, wsc, None, ALU.mult)
                        elif t < 8:
                            nc.vector.scalar_tensor_tensor(
                                yv, src, wsc, yv, ALU.mult, ALU.add)
                        else:
                            nc.vector.scalar_tensor_tensor(
                                yv, src, wsc, yv, ALU.mult, ALU.add,
                                accum_out=s1[:, u, 0:1])

                def emit_sq1_op(ysl, slot, eng, nm, nelem):
                    scr = p1.tile([128, PX], BF16, tag="sqscr", bufs=2,
                                  name=nm)
                    if eng == 'v':
                        nc.vector.scalar_tensor_tensor(
                            scr[:, 0:nelem], ysl, 1.0, ysl, ALU.mult, ALU.mult,
                            accum_out=q1[:, slot:slot + 1])
                    else:
                        nc.scalar.activation(
                            scr[:, 0:nelem], ysl, AF.Square,
                            accum_out=q1[:, slot:slot + 1])

                def emit_sq(img, cb):
                    u = cb * BL + img
                    eng = SQ1_ENG[(img, cb)]
                    if (img, cb) == PE_UNITS[-1]:
                        # tail unit: chunked so the AR1 trigger follows the
                        # last eviction by ~1us instead of a full square op
                        yr = y_t[:, cb, img, 0:48, :]
                        emit_sq1_op(yr, u, eng, f"sqm_{img}_{cb}", 48 * W)
                        yt2 = y_t[:, cb, img, 48:56, :]
                        emit_sq1_op(yt2, 8, eng, f"sqt_{img}_{cb}", 8 * W)
                    else:
                        emit_sq1_op(y_t[:, cb, img, :, :], u, eng,
                                    f"sqscr_{img}_{cb}", PX)

                # loads for the pipeline heads; PE weights for P2 queued after
                emit_load(*PE_UNITS[0], False)
                emit_load(*DVE_UNITS[0], True)
                emit_pe_unit(*PE_UNITS[0])
                emit_dve_unit(*DVE_UNITS[0])
                nc.sync.dma_start(par_sb[:], par_ext[:])
                nc.sync.dma_start(wtb_sb[:], wtb_ext[:])
                nc.sync.dma_start(wtf_sb[:], wtf_ext[:])
                for du in DVE_UNITS[1:]:
                    emit_load(*du, True)
                    emit_dve_unit(*du)
                for pu in PE_UNITS[1:]:
                    emit_load(*pu, False)
                    emit_pe_unit(*pu)
                for u in SQ1_ORDER:
                    emit_sq(*u)

            # ---- BN1 stats: single all-reduce (the collective chain is
            # gated by the warmup + launch skew anyway; two serial ARs cost
            # more than one slightly-later one) ----
            arA = pp.tile([128, 4], F32, tag="arA")
            nc.vector.tensor_reduce(arA[:, 0:1], s1[:, 0:4, :], axis=AX.XY, op=ALU.add)
            nc.vector.tensor_reduce(arA[:, 1:2], s1[:, 4:8, :], axis=AX.XY, op=ALU.add)
            nc.vector.tensor_reduce(arA[:, 2:3], q1[:, 0:4], axis=AX.X, op=ALU.add)
            nc.vector.tensor_reduce(arA[:, 3:4], q1[:, 4:9], axis=AX.X, op=ALU.add)

            arA_in = dram.tile([128, 4], F32)
            arA_out = dram.tile([128, 4], F32, addr_space="Shared")
            nc.sync.dma_start(arA_in[:], arA[:])
            nc.gpsimd.collective_compute(
                "AllReduce", ALU.add, replica_groups=[list(range(N_CORES))],
                ins=[arA_in[:].opt()], outs=[arA_out[:].opt()])
            gs1 = pp.tile([128, 4], F32, tag="gs1")
            nc.sync.dma_start(gs1[:], arA_out[:])

            def finalize_bn(sums, sqs, g_sl, b_sl, a_sb, c_sb, ncb, tg):
                mean = pp.tile([128, ncb], F32, tag=tg + "m")
                ex2 = pp.tile([128, ncb], F32, tag=tg + "e")
                var = pp.tile([128, ncb], F32, tag=tg + "v")
                std = pp.tile([128, ncb], F32, tag=tg + "s")
                rstd = pp.tile([128, ncb], F32, tag=tg + "r")
                tmp = pp.tile([128, ncb], F32, tag=tg + "t")
                inv = 1.0 / COUNT
                nc.vector.tensor_scalar_mul(mean[:], sums, inv)
                nc.vector.tensor_scalar_mul(ex2[:], sqs, inv)
                nc.vector.tensor_tensor(tmp[:], mean[:], mean[:], ALU.mult)
                nc.vector.tensor_tensor(var[:], ex2[:], tmp[:], ALU.subtract)
                nc.scalar.activation(std[:], var[:], AF.Sqrt, bias=epsb[:])
                nc.vector.reciprocal(rstd[:], std[:])
                nc.vector.tensor_tensor(a_sb[:], rstd[:], g_sl, ALU.mult)
                nc.vector.tensor_tensor(tmp[:], a_sb[:], mean[:], ALU.mult)
                nc.vector.tensor_tensor(c_sb[:], b_sl, tmp[:], ALU.subtract)

            finalize_bn(gs1[:, 0:2], gs1[:, 2:4], par_sb[:, 0:2], par_sb[:, 2:4],
                        a1, c1, NCB1, "f1")

            # z lives P2..P3 only; its pool opens after the P1 pools close
            zstack = contextlib.ExitStack()
            zp = zstack.enter_context(tc.tile_pool(name="zp", bufs=1))
            z_t = zp.tile([128, BL, NCB2, PX], BF16, tag="z")

            # ================= P2: relu-normalize, 1x1 GEMM, BN2 stats =======
            with (
                tc.tile_pool(name="p2sb", bufs=1) as p2,
                tc.tile_pool(name="p2ps", bufs=1, space="PSUM") as p2ps,
                nc.named_scope("P2_gemm"),
            ):
                # yh = relu(a1*y + c1) IN PLACE over y, emitted upfront.
                # img0's pair is split ACT/DVE so the first GEMM starts ~6us
                # after finalize instead of waiting on a serial ACT chain.
                def emit_yh(img, cb, r0, r1, slot):
                    ysl = y_t[:, cb, img, r0:r1, :]
                    if YH_ENG[(img, cb)] == 'v':
                        nc.vector.tensor_scalar(
                            ysl, ysl, a1[:, cb:cb + 1], c1[:, cb:cb + 1],
                            ALU.mult, ALU.add)
                        nc.vector.tensor_scalar(
                            ysl, ysl, 0.0, 0.0, ALU.max, ALU.add,
                            accum_out=s2s[:, cb, slot:slot + 1])
                    else:
                        nc.scalar.activation(
                            ysl, ysl, AF.Relu,
                            bias=c1[:, cb:cb + 1], scale=a1[:, cb:cb + 1],
                            accum_out=s2s[:, cb, slot:slot + 1])

                # img0 yh in halves so the first GEMM starts ~1.5us sooner
                for cb in range(NCB1):
                    emit_yh(0, cb, 0, 28, 0)
                for cb in range(NCB1):
                    emit_yh(0, cb, 28, 56, BL)
                for img in range(1, BL):
                    for cb in range(NCB1):
                        emit_yh(img, cb, 0, H, img)

                sv = p2.tile([128, NCB1], F32, tag="sv")
                sq_i = 0
                ev_i = 0
                for img in range(BL):
                    for q in range(NQ):
                        ps = p2ps.tile([128, 4, 512], F32, tag="ps2", bufs=2,
                                       name=f"ps2_{img}_{q}")
                        for ob in range(NCB2):
                            for cb in range(NCB1):
                                nc.tensor.matmul(
                                    ps[:, ob, 0:QW],
                                    wtb_sb[:, cb, ob * 128:(ob + 1) * 128],
                                    y_t[:, cb, img, q * 8:(q + 1) * 8, :],
                                    start=(cb == 0), stop=(cb == NCB1 - 1))
                        zdst = z_t[:, img, :, q * QW:(q + 1) * QW]
                        if ev_i % 2 == 0:
                            nc.vector.tensor_scalar(zdst, ps[:, 0:4, 0:QW],
                                                    1.0, None, ALU.mult)
                        else:
                            nc.scalar.activation(zdst, ps[:, 0:4, 0:QW], AF.Copy)
                        ev_i += 1

                    def emit_sq2(ob, zsl, slot, eng, nm):
                        scr = p2.tile([128, PX], BF16, tag="sq2scr", bufs=2,
                                      name=nm)
                        if eng == 'v':
                            nc.vector.scalar_tensor_tensor(
                                scr[:, 0:zsl.free_size()], zsl, 1.0, zsl,
                                ALU.mult, ALU.mult,
                                accum_out=q2[:, ob, slot:slot + 1])
                        else:
                            nc.scalar.activation(
                                scr[:, 0:zsl.free_size()], zsl, AF.Square,
                                accum_out=q2[:, ob, slot:slot + 1])

                    if img < 3:
                        for ob in range(NCB2):
                            emit_sq2(ob, z_t[:, img, ob, :], img,
                                     SQ2_PAT[sq_i], f"zs_{img}_{ob}")
                            sq_i += 1
                        if img == 0:
                            # keepalive: keeps the CC cores hot between AR1
                            # and AR2 so AR2's pickup latency stays small
                            w_out2 = dram.tile([128, 4], F32,
                                               addr_space="Shared")
                            nc.gpsimd.collective_compute(
                                "AllReduce", ALU.add,
                                replica_groups=[list(range(N_CORES))],
                                ins=[w_in[:].opt()], outs=[w_out2[:].opt()])
                    else:
                        for ob in range(NCB2):
                            emit_sq2(ob, z_t[:, 3, ob, 0:6 * QW], 3,
                                     SQ3_BIG[ob], f"zs3m_{ob}")
                        for ob in range(NCB2):
                            emit_sq2(ob, z_t[:, 3, ob, 6 * QW:PX], 4,
                                     SQ3_TAIL[ob], f"zs3t_{ob}")
                    if img == 2:
                        # channel sums of z via linearity: sum2 = W_f32 @
                        # rowsum(yh); emitted here so PE reaches it after
                        # img2's GEMM (sv long since ready) and AR2a can
                        # fire under img3's work.
                        for cb in range(NCB1):
                            nc.vector.tensor_reduce(
                                sv[:, cb:cb + 1], s2s[:, cb, :],
                                axis=AX.X, op=ALU.add)
                        ps_s = p2ps.tile([128, 4, 512], F32, tag="ps2",
                                         bufs=2, name="ps_s")
                        for ob in range(NCB2):
                            for cb in range(NCB1):
                                nc.tensor.matmul(
                                    ps_s[:, ob, 0:1],
                                    wtf_sb[:, cb, ob * 128:(ob + 1) * 128],
                                    sv[:, cb:cb + 1],
                                    start=(cb == 0), stop=(cb == NCB1 - 1))
                        nc.vector.tensor_scalar(sum2[:], ps_s[:, 0:4, 0:1],
                                                1.0, None, ALU.mult)

            # ---- AR2: one collective (sum2 + all squares); its trigger
            # chain is ~4us after the last eviction thanks to the img3
            # square chunking ----
            ar2 = pp.tile([128, 8], F32, tag="ar2")
            nc.vector.tensor_copy(ar2[:, 0:4], sum2[:])
            nc.vector.tensor_reduce(ar2[:, 4:8], q2[:], axis=AX.X, op=ALU.add)
            ar2_in = dram.tile([128, 8], F32)
            ar2_out = dram.tile([128, 8], F32, addr_space="Shared")
            nc.sync.dma_start(ar2_in[:], ar2[:])
            nc.gpsimd.collective_compute(
                "AllReduce", ALU.add, replica_groups=[list(range(N_CORES))],
                ins=[ar2_in[:].opt()], outs=[ar2_out[:].opt()])
            gA2 = pp.tile([128, 8], F32, tag="gA2")
            nc.sync.dma_start(gA2[:], ar2_out[:])

            finalize_bn(gA2[:, 0:4], gA2[:, 4:8], par_sb[:, 4:8], par_sb[:, 8:12],
                        a2, c2, NCB2, "f2")

            # ================= P3: BN2 affine + relu + store =================
            with tc.tile_pool(name="p3sb", bufs=1) as p3, nc.named_scope("P3_out"):
                pi = 0
                for img in range(BL):
                    for ob in range(NCB2):
                        ost = p3.tile([128, PX], BF16, tag="ost", bufs=4,
                                      name=f"ost_{img}_{ob}")
                        zsl = z_t[:, img, ob, :]
                        if P3_PAT[pi] == 'v':
                            nc.vector.tensor_scalar(
                                ost[:], zsl, a2[:, ob:ob + 1], c2[:, ob:ob + 1],
                                ALU.mult, ALU.add)
                            nc.vector.tensor_scalar(
                                ost[:], ost[:], 0.0, None, ALU.max)
                        else:
                            nc.scalar.activation(
                                ost[:], zsl, AF.Relu,
                                bias=c2[:, ob:ob + 1], scale=a2[:, ob:ob + 1])
                        nc.sync.dma_start(out_ext[img, ob], ost[:])
                        pi += 1
            zstack.close()

    nc.compile()
    return nc


_NC_CACHE = None


def _get_nc():
    global _NC_CACHE
    if _NC_CACHE is None:
        _NC_CACHE = build()
    return _NC_CACHE


def _prep_in_maps(inputs):
    bf16 = mybir.dt.np(BF16)
    x = np.asarray(inputs["x"], dtype=np.float32)
    # reflect pad; data at rows 0..57, col k of the padded image sits at
    # buf col k+1 (xp) / k+2 (xs) so every flat tap offset is 4B-aligned
    xpad = np.pad(x, ((0, 0), (0, 0), (1, 1), (1, 1)), mode='reflect')
    xpb = xpad.astype(bf16)
    xbuf = np.zeros((B, C1, HP, WD), dtype=bf16)
    xbuf[:, :, :, 1:59] = xpb

    dww = np.asarray(inputs["dw_w"], dtype=np.float32).reshape(C1, 9)
    diag = np.zeros((128, NCB1, 9, 128), dtype=np.float32)
    k = np.arange(128)
    for cb in range(NCB1):
        for t in range(9):
            diag[k, cb, t, k] = dww[cb * 128 + k, t]
    diag = diag.astype(bf16)
    dwt = np.ascontiguousarray(
        dww.reshape(NCB1, 128, 9).transpose(1, 0, 2))  # [128, cb, 9]

    pw = np.asarray(inputs["pw_w"], dtype=np.float32)   # [C2, C1]
    wt = np.ascontiguousarray(
        pw.T.reshape(NCB1, 128, C2).transpose(1, 0, 2))  # [128, cb, C2]
    wtb = wt.astype(bf16)
    wtf = wt.astype(np.float32)

    par = np.zeros((128, 12), dtype=np.float32)
    par[:, 0:2] = np.asarray(inputs["g1"], np.float32).reshape(NCB1, 128).T
    par[:, 2:4] = np.asarray(inputs["b1"], np.float32).reshape(NCB1, 128).T
    par[:, 4:8] = np.asarray(inputs["g2"], np.float32).reshape(NCB2, 128).T
    par[:, 8:12] = np.asarray(inputs["b2"], np.float32).reshape(NCB2, 128).T

    in_maps = []
    for core in range(N_CORES):
        xc = xbuf[core * BL:(core + 1) * BL].reshape(BL, NCB1, 128, HP, WD)
        in_maps.append({
            "x": np.ascontiguousarray(xc),
            "diag": diag,
            "dwt": dwt,
            "wtb": wtb,
            "wtf": wtf,
            "par": par,
        })
    return in_maps


def run(inputs, trace=False):
    nc = _get_nc()
    in_maps = _prep_in_maps(inputs)
    res = run_bass_kernel_spmd(nc, in_maps, list(range(N_CORES)), trace=trace)
    outs = []
    for i in range(N_CORES):
        o = np.asarray(res.results[i]["out"]).astype(np.float32)
        outs.append(o.reshape(BL, C2, H, W))
    return np.concatenate(outs, axis=0), res


def kernel(**inputs):
    out, _ = run(inputs, trace=False)
    return out


# revision 54
# speedup vs baseline: 1.0042x; 1.0042x over previous
"""Trainium2 Bass kernel for DepthSepConv2d (depthwise 3x3 reflect-pad conv +
sync-BN + ReLU + 1x1 conv + sync-BN + ReLU), data-parallel over batch on 8
NeuronCores.

Self-contained: hardcodes all shapes; host-side code reflect-pads + converts
to bf16, runs the SPMD NEFF, and reassembles the f32 output.

Per core (4 images, 256ch in / 512ch out, 56x56):
  P1: depthwise 3x3 — 5 units on PE (per-tap diagonal matmuls into PSUM,
      2-bank-group ACT evictions with accum stats), 3 units on DVE using
      FLAT shifted taps: all 9 taps are flat step-1 bf16 STT ops over the
      whole padded image (hits the DVE 2x packed mode; pad positions carry
      garbage that is never read), with a column-shifted copy of x supplying
      4-byte-aligned sources for odd shifts. ACT extracts the valid window
      (+BN sum); squares run on GPSIMD.
  AR1a/AR1b: split sync-BN all-reduce; AR1a (7 units) hides under the tail
      unit's compute; one warmup collective at t=0 absorbs core launch skew.
  P2: yh = relu(a1*y+c1) computed IN PLACE over y on ACT; 1x1 GEMM from the
      same tiles; one batched 4-bank PSUM eviction per tile on DVE (no
      accumulators: BN2 channel sums come from a tiny f32 matmul on the yh
      row sums); squares split GPSIMD/DVE/ACT. s-matmuls emitted mid-GEMM
      so AR2a hides under the last image's GEMM.
  P3: ACT/DVE normalize + ReLU, bf16 output DMA (host converts to f32).
"""

import contextlib

import numpy as np

from concourse import bacc, mybir, tile
from concourse.bass_utils import run_bass_kernel_spmd

N_CORES = 8
B, C1, C2, H, W = 32, 256, 512, 56, 56
BL = B // N_CORES            # images per core
PX = H * W                   # 3136
HP, WD = H + 2, W + 4        # padded rows 58, padded row width 60
XF = HP * WD                 # 3480 flat padded image
XFP = XF + 4                 # flat tile size (tap reads run 2 past the end)
YF = 56 * WD                 # 3360 flat tap-output rows
NN = 3358                    # flat tap span (covers extracted region)
QW = 448                     # pixel tile (8 rows), 7 per image
NQ = PX // QW                # 7
NCB1 = C1 // 128             # 2
NCB2 = C2 // 128             # 4
COUNT = B * PX               # global BN reduction count
EPS = 1e-5

F32 = mybir.dt.float32
BF16 = mybir.dt.bfloat16
AF = mybir.ActivationFunctionType
ALU = mybir.AluOpType
AX = mybir.AxisListType

TAPS = [(dh, dw) for dh in range(3) for dw in range(3)]

# unit = (img, cb); stats slot u = cb*BL + img (cb-major so the partial
# all-reduce slices contiguously; the tail unit must own the last slot).
DVE_UNITS = [(0, 0), (0, 1)]
PE_UNITS = [(1, 0), (1, 1), (2, 0), (2, 1), (3, 0), (3, 1)]
N_DVE = len(DVE_UNITS)

QGROUPS = [(0, 1), (2, 3), (4, 5), (6,)]

# engine choices: 'v' = DVE, 'a' = ACT, 'g' = GPSIMD (copies only).
# Emission order follows expected y completion so queues never head-block;
# the tail unit's square stays on ACT so it cannot delay AR1a.
SQ1_ENG = {u: 'a' for u in DVE_UNITS + PE_UNITS}
SQ1_ENG[(0, 0)] = 'v'
SQ1_ENG[(0, 1)] = 'v'
SQ1_ORDER = [(1, 0), (1, 1), (0, 0), (2, 0), (2, 1), (0, 1), (3, 0), (3, 1)]
SQ2_PAT = ['v', 'a', 'v', 'a', 'a', 'v', 'a', 'v',
           'v', 'a', 'v', 'a']                 # per (img, ob), img 0..2
SQ3_BIG = ['v', 'a', 'v', 'a']                 # img3 main chunk (q0..5)
SQ3_TAIL = ['a', 'v', 'a', 'v']                # img3 tail chunk (q6)
# yh engine per (img, cb): DVE covers the early images (it idles at P2
# start), keeping the ACT chain short enough that GEMM never waits.
YH_ENG = {(0, 0): 'a', (0, 1): 'v', (1, 0): 'a', (1, 1): 'a',
          (2, 0): 'a', (2, 1): 'a', (3, 0): 'a', (3, 1): 'a'}
P3_PAT = ['a', 'v'] * 8


def build():
    nc = bacc.Bacc(None, target_bir_lowering=False, debug=False)

    x_ext = nc.declare_dram_parameter("x", [BL, NCB1, 128, HP, WD], BF16, isOutput=False)
    diag_ext = nc.declare_dram_parameter("diag", [128, NCB1, 9, 128], BF16, isOutput=False)
    dwt_ext = nc.declare_dram_parameter("dwt", [128, NCB1, 9], F32, isOutput=False)
    wtb_ext = nc.declare_dram_parameter("wtb", [128, NCB1, C2], BF16, isOutput=False)
    wtf_ext = nc.declare_dram_parameter("wtf", [128, NCB1, C2], F32, isOutput=False)
    par_ext = nc.declare_dram_parameter("par", [128, 12], F32, isOutput=False)
    out_ext = nc.declare_dram_parameter("out", [BL, NCB2, 128, PX], BF16, isOutput=True)

    with tile.TileContext(nc) as tc:
        with (
            tc.tile_pool(name="persist", bufs=1) as pp,
            tc.tile_pool(name="dram", bufs=1, space="DRAM") as dram,
        ):
            # ---- persistent tiles ----
            y_t = pp.tile([128, NCB1, BL, H, W], BF16, tag="y")

            dwt_sb = pp.tile([128, NCB1, 9], F32, tag="dwt")
            wtb_sb = pp.tile([128, NCB1, C2], BF16, tag="wtb")
            wtf_sb = pp.tile([128, NCB1, C2], F32, tag="wtf")
            par_sb = pp.tile([128, 12], F32, tag="par")

            s1 = pp.tile([128, 2 * BL, 4], F32, tag="s1")     # dw sums, slot u=cb*BL+img
            q1 = pp.tile([128, 2 * BL + 1], F32, tag="q1")    # dw sumsq; the tail
            # unit splits its square into slots 7 (q0-5) + 8 (q6 tail)
            s2s = pp.tile([128, NCB1, BL + 1], F32, tag="s2s")  # yh row sums
            # (slot 4 = second half of img0, whose yh is split for latency)
            q2 = pp.tile([128, NCB2, 5], F32, tag="q2")       # z sumsq slots:
            # 0..2 = img0..2; 3 = img3 q0-5; 4 = img3 q6 (split so the AR2b
            # inputs are ready ~2us after the last eviction)
            sum2 = pp.tile([128, NCB2], F32, tag="sum2")

            a1 = pp.tile([128, NCB1], F32, tag="a1")
            c1 = pp.tile([128, NCB1], F32, tag="c1")
            a2 = pp.tile([128, NCB2], F32, tag="a2")
            c2 = pp.tile([128, NCB2], F32, tag="c2")
            epsb = pp.tile([128, 1], F32, tag="epsb")

            # warmup collective buffers (emitted in P1 after the first loads
            # so x/diag take the head of the DMA queue)
            wsb = pp.tile([128, 4], F32, tag="wsb")
            nc.vector.memset(wsb[:], 0.0)
            w_in = dram.tile([128, 4], F32)
            w_out1 = dram.tile([128, 4], F32, addr_space="Shared")

            nc.vector.memset(epsb[:], EPS)
            nc.vector.memset(s1[:], 0.0)

            # ================= P1: depthwise conv + BN1 stats =================
            with (
                tc.tile_pool(name="p1sb", bufs=1) as p1,
                tc.tile_pool(name="p1ps", bufs=1, space="PSUM") as p1ps,
                nc.named_scope("P1_dwconv"),
            ):
                diag_sb = p1.tile([128, NCB1, 9, 128], BF16, tag="diag")
                nc.sync.dma_start(diag_sb[:], diag_ext[:])
                nc.sync.dma_start(dwt_sb[:], dwt_ext[:])

                xp_t = {}

                def emit_load(img, cb, dve):
                    # two half-loads so the first matmuls start ~2us sooner
                    xp = p1.tile([128, HP, WD], BF16,
                                 tag="xpv" if dve else "xp", bufs=2,
                                 name=f"xp_{img}_{cb}")
                    nc.sync.dma_start(xp[:, 0:29, :], x_ext[img, cb, :, 0:29, :])
                    nc.sync.dma_start(xp[:, 29:HP, :], x_ext[img, cb, :, 29:HP, :])
                    xp_t[(img, cb)] = xp

                def emit_pe_unit(img, cb):
                    u = cb * BL + img
                    xp = xp_t[(img, cb)]
                    for g, qs in enumerate(QGROUPS):
                        ps = p1ps.tile([128, 2, 512], F32, tag="dps", bufs=4,
                                       name=f"dps_{img}_{cb}_{g}")
                        for qi, q in enumerate(qs):
                            for t, (dh, dw) in enumerate(TAPS):
                                rhs = xp[:, q * 8 + dh: q * 8 + dh + 8,
                                         dw + 1: dw + 57]
                                nc.tensor.matmul(
                                    ps[:, qi, 0:QW], diag_sb[:, cb, t, :], rhs,
                                    start=(t == 0), stop=(t == 8))
                        r0 = qs[0] * 8
                        nr = len(qs) * 8
                        nc.scalar.activation(
                            y_t[:, cb, img, r0:r0 + nr, :],
                            ps[:, 0:len(qs), 0:QW], AF.Copy,
                            accum_out=s1[:, u, g:g + 1])

                def emit_dve_unit(img, cb):
                    u = cb * BL + img
                    xp = xp_t[(img, cb)]
                    yv = y_t[:, cb, img, :, :]
                    for t, (dh, dw) in enumerate(TAPS):
                        src = xp[:, dh:dh + H, dw + 1:dw + 57]
                        wsc = dwt_sb[:, cb, t:t + 1]
                        if t == 0:
                            nc.vector.tensor_scalar(yv, src, wsc, None, ALU.mult)
                        elif t < 8:
                            nc.vector.scalar_tensor_tensor(
                                yv, src, wsc, yv, ALU.mult, ALU.add)
                        else:
                            nc.vector.scalar_tensor_tensor(
                                yv, src, wsc, yv, ALU.mult, ALU.add,
                                accum_out=s1[:, u, 0:1])

                def emit_sq1_op(ysl, slot, eng, nm, nelem):
                    scr = p1.tile([128, PX], BF16, tag="sqscr", bufs=2,
                                  name=nm)
                    if eng == 'v':
                        nc.vector.scalar_tensor_tensor(
                            scr[:, 0:nelem], ysl, 1.0, ysl, ALU.mult, ALU.mult,
                            accum_out=q1[:, slot:slot + 1])
                    else:
                        nc.scalar.activation(
                            scr[:, 0:nelem], ysl, AF.Square,
                            accum_out=q1[:, slot:slot + 1])

                def emit_sq(img, cb):
                    u = cb * BL + img
                    eng = SQ1_ENG[(img, cb)]
                    if (img, cb) == PE_UNITS[-1]:
                        # tail unit: chunked so the AR1 trigger follows the
                        # last eviction by ~1us instead of a full square op
                        yr = y_t[:, cb, img, 0:48, :]
                        emit_sq1_op(yr, u, eng, f"sqm_{img}_{cb}", 48 * W)
                        yt2 = y_t[:, cb, img, 48:56, :]
                        emit_sq1_op(yt2, 8, eng, f"sqt_{img}_{cb}", 8 * W)
                    else:
                        emit_sq1_op(y_t[:, cb, img, :, :], u, eng,
                                    f"sqscr_{img}_{cb}", PX)

                # loads for the pipeline heads; PE weights for P2 queued after
                emit_load(*PE_UNITS[0], False)
                emit_load(*DVE_UNITS[0], True)
                # warmup collective (absorbs cross-core launch skew / CC boot)
                nc.sync.dma_start(w_in[:], wsb[:])
                nc.gpsimd.collective_compute(
                    "AllReduce", ALU.add,
                    replica_groups=[list(range(N_CORES))],
                    ins=[w_in[:].opt()], outs=[w_out1[:].opt()],
                )
                emit_pe_unit(*PE_UNITS[0])
                emit_dve_unit(*DVE_UNITS[0])
                nc.sync.dma_start(par_sb[:], par_ext[:])
                nc.sync.dma_start(wtb_sb[:], wtb_ext[:])
                nc.sync.dma_start(wtf_sb[:], wtf_ext[:])
                for du in DVE_UNITS[1:]:
                    emit_load(*du, True)
                    emit_dve_unit(*du)
                for pu in PE_UNITS[1:]:
                    emit_load(*pu, False)
                    emit_pe_unit(*pu)
                for u in SQ1_ORDER:
                    emit_sq(*u)

            # ---- BN1 stats: single all-reduce (the collective chain is
            # gated by the warmup + launch skew anyway; two serial ARs cost
            # more than one slightly-later one) ----
            arA = pp.tile([128, 4], F32, tag="arA")
            nc.vector.tensor_reduce(arA[:, 0:1], s1[:, 0:4, :], axis=AX.XY, op=ALU.add)
            nc.vector.tensor_reduce(arA[:, 1:2], s1[:, 4:8, :], axis=AX.XY, op=ALU.add)
            nc.vector.tensor_reduce(arA[:, 2:3], q1[:, 0:4], axis=AX.X, op=ALU.add)
            nc.vector.tensor_reduce(arA[:, 3:4], q1[:, 4:9], axis=AX.X, op=ALU.add)

            arA_in = dram.tile([128, 4], F32)
            arA_out = dram.tile([128, 4], F32, addr_space="Shared")
            nc.sync.dma_start(arA_in[:], arA[:])
            nc.gpsimd.collective_compute(
                "AllReduce", ALU.add, replica_groups=[list(range(N_CORES))],
                ins=[arA_in[:].opt()], outs=[arA_out[:].opt()])
            gs1 = pp.tile([128, 4], F32, tag="gs1")
            nc.sync.dma_start(gs1[:], arA_out[:])

            def finalize_bn(sums, sqs, g_sl, b_sl, a_sb, c_sb, ncb, tg):
                mean = pp.tile([128, ncb], F32, tag=tg + "m")
                ex2 = pp.tile([128, ncb], F32, tag=tg + "e")
                var = pp.tile([128, ncb], F32, tag=tg + "v")
                std = pp.tile([128, ncb], F32, tag=tg + "s")
                rstd = pp.tile([128, ncb], F32, tag=tg + "r")
                tmp = pp.tile([128, ncb], F32, tag=tg + "t")
                inv = 1.0 / COUNT
                nc.vector.tensor_scalar_mul(mean[:], sums, inv)
                nc.vector.tensor_scalar_mul(ex2[:], sqs, inv)
                nc.vector.tensor_tensor(tmp[:], mean[:], mean[:], ALU.mult)
                nc.vector.tensor_tensor(var[:], ex2[:], tmp[:], ALU.subtract)
                nc.scalar.activation(std[:], var[:], AF.Sqrt, bias=epsb[:])
                nc.vector.reciprocal(rstd[:], std[:])
                nc.vector.tensor_tensor(a_sb[:], rstd[:], g_sl, ALU.mult)
                nc.vector.tensor_tensor(tmp[:], a_sb[:], mean[:], ALU.mult)
                nc.vector.tensor_tensor(c_sb[:], b_sl, tmp[:], ALU.subtract)

            finalize_bn(gs1[:, 0:2], gs1[:, 2:4], par_sb[:, 0:2], par_sb[:, 2:4],
                        a1, c1, NCB1, "f1")

            # z lives P2..P3 only; its pool opens after the P1 pools close
            zstack = contextlib.ExitStack()
            zp = zstack.enter_context(tc.tile_pool(name="zp", bufs=1))
            z_t = zp.tile([128, BL, NCB2, PX], BF16, tag="z")

            # ================= P2: relu-normalize, 1x1 GEMM, BN2 stats =======
            with (
                tc.tile_pool(name="p2sb", bufs=1) as p2,
                tc.tile_pool(name="p2ps", bufs=1, space="PSUM") as p2ps,
                nc.named_scope("P2_gemm"),
            ):
                # yh = relu(a1*y + c1) IN PLACE over y, emitted upfront.
                # img0's pair is split ACT/DVE so the first GEMM starts ~6us
                # after finalize instead of waiting on a serial ACT chain.
                def emit_yh(img, cb, r0, r1, slot):
                    ysl = y_t[:, cb, img, r0:r1, :]
                    if YH_ENG[(img, cb)] == 'v':
                        nc.vector.tensor_scalar(
                            ysl, ysl, a1[:, cb:cb + 1], c1[:, cb:cb + 1],
                            ALU.mult, ALU.add)
                        nc.vector.tensor_scalar(
                            ysl, ysl, 0.0, 0.0, ALU.max, ALU.add,
                            accum_out=s2s[:, cb, slot:slot + 1])
                    else:
                        nc.scalar.activation(
                            ysl, ysl, AF.Relu,
                            bias=c1[:, cb:cb + 1], scale=a1[:, cb:cb + 1],
                            accum_out=s2s[:, cb, slot:slot + 1])

                # img0 yh in halves so the first GEMM starts ~1.5us sooner
                for cb in range(NCB1):
                    emit_yh(0, cb, 0, 28, 0)
                for cb in range(NCB1):
                    emit_yh(0, cb, 28, 56, BL)
                for img in range(1, BL):
                    for cb in range(NCB1):
                        emit_yh(img, cb, 0, H, img)

                sv = p2.tile([128, NCB1], F32, tag="sv")
                sq_i = 0
                ev_i = 0
                for img in range(BL):
                    for q in range(NQ):
                        ps = p2ps.tile([128, 4, 512], F32, tag="ps2", bufs=2,
                                       name=f"ps2_{img}_{q}")
                        for ob in range(NCB2):
                            for cb in range(NCB1):
                                nc.tensor.matmul(
                                    ps[:, ob, 0:QW],
                                    wtb_sb[:, cb, ob * 128:(ob + 1) * 128],
                                    y_t[:, cb, img, q * 8:(q + 1) * 8, :],
                                    start=(cb == 0), stop=(cb == NCB1 - 1))
                        zdst = z_t[:, img, :, q * QW:(q + 1) * QW]
                        if ev_i % 2 == 1:
                            nc.vector.tensor_scalar(zdst, ps[:, 0:4, 0:QW],
                                                    1.0, None, ALU.mult)
                        else:
                            nc.scalar.activation(zdst, ps[:, 0:4, 0:QW], AF.Copy)
                        ev_i += 1

                    def emit_sq2(ob, zsl, slot, eng, nm):
                        scr = p2.tile([128, PX], BF16, tag="sq2scr", bufs=2,
                                      name=nm)
                        if eng == 'v':
                            nc.vector.scalar_tensor_tensor(
                                scr[:, 0:zsl.free_size()], zsl, 1.0, zsl,
                                ALU.mult, ALU.mult,
                                accum_out=q2[:, ob, slot:slot + 1])
                        else:
                            nc.scalar.activation(
                                scr[:, 0:zsl.free_size()], zsl, AF.Square,
                                accum_out=q2[:, ob, slot:slot + 1])

                    if img < 3:
                        for ob in range(NCB2):
                            emit_sq2(ob, z_t[:, img, ob, :], img,
                                     SQ2_PAT[sq_i], f"zs_{img}_{ob}")
                            sq_i += 1
                        if img == 1:
                            # keepalive: an AR whose input DMA depends on
                            # img1's z, so its doorbell rings mid-P2 and the
                            # CC cores stay hot until AR2 (small pickup lat)
                            ka_in = dram.tile([128, 4], F32)
                            w_out2 = dram.tile([128, 4], F32,
                                               addr_space="Shared")
                            nc.sync.dma_start(ka_in[:],
                                              z_t[:, 1, 0:1, 0:8].bitcast(F32))
                            nc.gpsimd.collective_compute(
                                "AllReduce", ALU.add,
                                replica_groups=[list(range(N_CORES))],
                                ins=[ka_in[:].opt()], outs=[w_out2[:].opt()])
                    else:
                        for ob in range(NCB2):
                            emit_sq2(ob, z_t[:, 3, ob, 0:6 * QW], 3,
                                     SQ3_BIG[ob], f"zs3m_{ob}")
                        for ob in range(NCB2):
                            emit_sq2(ob, z_t[:, 3, ob, 6 * QW:PX], 4,
                                     SQ3_TAIL[ob], f"zs3t_{ob}")
                    if img == 2:
                        # channel sums of z via linearity: sum2 = W_f32 @
                        # rowsum(yh); emitted here so PE reaches it after
                        # img2's GEMM (sv long since ready) and AR2a can
                        # fire under img3's work.
                        for cb in range(NCB1):
                            nc.vector.tensor_reduce(
                                sv[:, cb:cb + 1], s2s[:, cb, :],
                                axis=AX.X, op=ALU.add)
                        ps_s = p2ps.tile([128, 4, 512], F32, tag="ps2",
                                         bufs=2, name="ps_s")
                        for ob in range(NCB2):
                            for cb in range(NCB1):
                                nc.tensor.matmul(
                                    ps_s[:, ob, 0:1],
                                    wtf_sb[:, cb, ob * 128:(ob + 1) * 128],
                                    sv[:, cb:cb + 1],
                                    start=(cb == 0), stop=(cb == NCB1 - 1))
                        nc.vector.tensor_scalar(sum2[:], ps_s[:, 0:4, 0:1],
                                                1.0, None, ALU.mult)

            # ---- AR2: one collective (sum2 + all squares); its trigger
            # chain is ~4us after the last eviction thanks to the img3
            # square chunking ----
            ar2 = pp.tile([128, 8], F32, tag="ar2")
            nc.vector.tensor_copy(ar2[:, 0:4], sum2[:])
            nc.vector.tensor_reduce(ar2[:, 4:8], q2[:], axis=AX.X, op=ALU.add)
            ar2_in = dram.tile([128, 8], F32)
            ar2_out = dram.tile([128, 8], F32, addr_space="Shared")
            nc.sync.dma_start(ar2_in[:], ar2[:])
            nc.gpsimd.collective_compute(
                "AllReduce", ALU.add, replica_groups=[list(range(N_CORES))],
                ins=[ar2_in[:].opt()], outs=[ar2_out[:].opt()])
            gA2 = pp.tile([128, 8], F32, tag="gA2")
            nc.sync.dma_start(gA2[:], ar2_out[:])

            finalize_bn(gA2[:, 0:4], gA2[:, 4:8], par_sb[:, 4:8], par_sb[:, 8:12],
                        a2, c2, NCB2, "f2")

            # ================= P3: BN2 affine + relu + store =================
            with tc.tile_pool(name="p3sb", bufs=1) as p3, nc.named_scope("P3_out"):
                def emit_p3(img, ob, n0, n1, eng, nm):
                    ost = p3.tile([128, PX], BF16, tag="ost", bufs=4, name=nm)
                    zsl = z_t[:, img, ob, n0:n1]
                    osl = ost[:, 0:n1 - n0]
                    if eng == 'v':
                        nc.vector.tensor_scalar(
                            osl, zsl, a2[:, ob:ob + 1], c2[:, ob:ob + 1],
                            ALU.mult, ALU.add)
                        nc.vector.tensor_scalar(osl, osl, 0.0, None, ALU.max)
                    else:
                        nc.scalar.activation(
                            osl, zsl, AF.Relu,
                            bias=c2[:, ob:ob + 1], scale=a2[:, ob:ob + 1])
                    nc.sync.dma_start(
                        out_ext[img, ob, :, n0:n1], osl)

                pi = 0
                for img in range(BL):
                    for ob in range(NCB2):
                        if img == 0 and ob < 2:
                            # halves: first output DMA starts ~1.6us sooner
                            emit_p3(img, ob, 0, PX // 2, P3_PAT[pi],
                                    f"ost_{img}_{ob}a")
                            emit_p3(img, ob, PX // 2, PX, P3_PAT[pi],
                                    f"ost_{img}_{ob}b")
                        else:
                            emit_p3(img, ob, 0, PX, P3_PAT[pi],
                                    f"ost_{img}_{ob}")
                        pi += 1
            zstack.close()

    nc.compile()
    return nc


_NC_CACHE = None


def _get_nc():
    global _NC_CACHE
    if _NC_CACHE is None:
        _NC_CACHE = build()
    return _NC_CACHE


def _prep_in_maps(inputs):
    bf16 = mybir.dt.np(BF16)
    x = np.asarray(inputs["x"], dtype=np.float32)
    # reflect pad; data at rows 0..57, col k of the padded image sits at
    # buf col k+1 (xp) / k+2 (xs) so every flat tap offset is 4B-aligned
    xpad = np.pad(x, ((0, 0), (0, 0), (1, 1), (1, 1)), mode='reflect')
    xpb = xpad.astype(bf16)
    xbuf = np.zeros((B, C1, HP, WD), dtype=bf16)
    xbuf[:, :, :, 1:59] = xpb

    dww = np.asarray(inputs["dw_w"], dtype=np.float32).reshape(C1, 9)
    diag = np.zeros((128, NCB1, 9, 128), dtype=np.float32)
    k = np.arange(128)
    for cb in range(NCB1):
        for t in range(9):
            diag[k, cb, t, k] = dww[cb * 128 + k, t]
    diag = diag.astype(bf16)
    dwt = np.ascontiguousarray(
        dww.reshape(NCB1, 128, 9).transpose(1, 0, 2))  # [128, cb, 9]

    pw = np.asarray(inputs["pw_w"], dtype=np.float32)   # [C2, C1]
    wt = np.ascontiguousarray(
        pw.T.reshape(NCB1, 128, C2).transpose(1, 0, 2))  # [128, cb, C2]
    wtb = wt.astype(bf16)
    wtf = wt.astype(np.float32)

    par = np.zeros((128, 12), dtype=np.float32)
    par[:, 0:2] = np.asarray(inputs["g1"], np.float32).reshape(NCB1, 128).T
    par[:, 2:4] = np.asarray(inputs["b1"], np.float32).reshape(NCB1, 128).T
    par[:, 4:8] = np.asarray(inputs["g2"], np.float32).reshape(NCB2, 128).T
    par[:, 8:12] = np.asarray(inputs["b2"], np.float32).reshape(NCB2, 128).T

    in_maps = []
    for core in range(N_CORES):
        xc = xbuf[core * BL:(core + 1) * BL].reshape(BL, NCB1, 128, HP, WD)
        in_maps.append({
            "x": np.ascontiguousarray(xc),
            "diag": diag,
            "dwt": dwt,
            "wtb": wtb,
            "wtf": wtf,
            "par": par,
        })
    return in_maps


def run(inputs, trace=False):
    nc = _get_nc()
    in_maps = _prep_in_maps(inputs)
    res = run_bass_kernel_spmd(nc, in_maps, list(range(N_CORES)), trace=trace)
    outs = []
    for i in range(N_CORES):
        o = np.asarray(res.results[i]["out"]).astype(np.float32)
        outs.append(o.reshape(BL, C2, H, W))
    return np.concatenate(outs, axis=0), res


def kernel(**inputs):
    out, _ = run(inputs, trace=False)
    return out
